# revision 1
# baseline (speedup 1.0000x reference)
"""Trainium2 Bass kernel for nn_MultiHeadedAttention_6416681140387.

Two-branch windowed video attention:
  x [8,256,96,96] -> 1x1 conv Q/K/V -> per-branch full attention over
  window-token features (branch0: 4x4 patches, d=2048, 2304 key tokens;
  branch1: 8x8 patches, d=8192, 576 key tokens) -> concat channels
  -> 3x3 conv + LeakyReLU(0.2).

Sharding: 8 cores = (video b in {0,1}) x (frame t in {0..3}). Each core
computes its full output frame [256,96,96]; K/V are recomputed per core from
its 4-frame video slice (no collectives). Host stacks the 8 frames.

Numerics: conv matmuls run in float32r (full-rate 4-byte PE mode); the
attention path (Q/K scores, P, V) runs in bf16 with fp32 PSUM accumulation.
Branch1 key tokens are padded 144->160 per frame so frame boundaries stay
32-aligned in the 128-partition PV tiling; padded scores are -1e30 -> P=0.
"""

import sys

if "/opt/trn_rl_repo" not in sys.path:
    sys.path.insert(0, "/opt/trn_rl_repo")

import math
from contextlib import ExitStack

import numpy as np

import concourse.bass as bass
import concourse.tile as tile
from concourse import bacc, mybir
from concourse.masks import make_identity

F32 = mybir.dt.float32
F32R = mybir.dt.float32r
BF16 = mybir.dt.bfloat16

T = 4
C = 256
H = W = 96
PIX = H * W
NCORES = 8

PSZ = [4, 8]
OHB = [24, 12]                  # token grid side per branch
NTF = [576, 144]                # real tokens per frame
NTFP = [576, 160]               # padded tokens per frame
NKP = [2304, 640]               # padded key tokens per video
NQ = [576, 144]                 # query tokens (one frame)
NCH = [16, 64]                  # d-chunks (psz^2)
SC = [1.0 / math.sqrt(2048.0), 1.0 / math.sqrt(8192.0)]
NQB = [[(0, 128), (128, 128), (256, 128), (384, 128), (512, 64)],
       [(0, 128), (128, 16)]]
NEG = -1.0e30

Exp = mybir.ActivationFunctionType.Exp
Identity = mybir.ActivationFunctionType.Identity


def _subpieces(br):
    """V/PT chunk tiles: list over tiles ti of list of sub-pieces
    (kf, ftok0, m, off). Partition offsets obey the PE col-group rule:
    off 0 -> m<=128, off 64 -> m<=64, off 32/96 -> m<=32."""
    ntiles = NKP[br] // 128
    out = []
    for ti in range(ntiles):
        lo, hi = ti * 128, ti * 128 + 128
        pieces = []
        for kf in range(T):
            f0 = kf * NTFP[br]
            a, b = max(lo, f0), min(hi, f0 + NTF[br])
            while a < b:
                off = a - lo
                cap = 128 - off if off == 0 else (64 if off == 64 else 32)
                m = min(b - a, cap)
                pieces.append((kf, a - f0, m, off))
                a += m
        out.append(pieces)
    return out


def _pad_rows(br, ti, pieces):
    """Partition ranges of V tile ti not covered by real tokens."""
    used = sorted((off, off + m) for (_, _, m, off) in pieces)
    gaps, pos = [], 0
    for a, b in used:
        if a > pos:
            gaps.append((pos, a))
        pos = b
    if pos < 128:
        gaps.append((pos, 128))
    return gaps


PHASES = {"A", "SM", "C0", "C1", "D"}


def build(nc):
    xv = nc.dram_tensor("xv", [T, C, PIX], F32R, kind="ExternalInput")
    xf = nc.dram_tensor("xf", [C, PIX], F32R, kind="ExternalInput")
    wqt = nc.dram_tensor("wqt", [C, C], F32R, kind="ExternalInput")
    wkt = nc.dram_tensor("wkt", [C, C], F32R, kind="ExternalInput")
    wvt = nc.dram_tensor("wvt", [C, C], F32R, kind="ExternalInput")
    wot = nc.dram_tensor("wot", [9, C, C], F32R, kind="ExternalInput")
    bq = nc.dram_tensor("bq", [C], F32, kind="ExternalInput")
    bk = nc.dram_tensor("bk", [C], F32, kind="ExternalInput")
    bv = nc.dram_tensor("bv", [C], F32, kind="ExternalInput")
    bo = nc.dram_tensor("bo", [C], F32, kind="ExternalInput")
    out = nc.dram_tensor("out", [C, PIX], F32, kind="ExternalOutput")

    alt = [0]

    def bias_copy_alt(dst, src, bias_ap):
        alt[0] ^= 1
        if alt[0]:
            nc.scalar.activation(out=dst, in_=src, func=Identity,
                                 bias=bias_ap, scale=1.0)
        else:
            nc.vector.tensor_scalar_add(dst, src, bias_ap)

    rr = [0]

    def copy_rr(dst, src):
        rr[0] = (rr[0] + 1) % 3
        if rr[0] == 0:
            nc.vector.tensor_copy(dst, src)
        elif rr[0] == 1:
            nc.scalar.copy(dst, src)
        else:
            nc.gpsimd.tensor_copy(dst, src)

    with tile.TileContext(nc, pool_alloc_mode="queue") as tc, ExitStack() as top:
        persist = top.enter_context(tc.tile_pool(name="persist", bufs=1))
        dramp = top.enter_context(tc.tile_pool(name="dram", bufs=1, space="DRAM"))

        wq_sb, wk_sb, wv_sb = [None, None], [None, None], [None, None]
        for name, dt_, lst in (("wq", wqt, wq_sb), ("wk", wkt, wk_sb),
                               ("wv", wvt, wv_sb)):
            for cb in range(2):
                t = persist.tile([128, C], F32R, name=f"{name}{cb}", tag=f"{name}{cb}")
                nc.sync.dma_start(out=t, in_=dt_.ap()[cb * 128:(cb + 1) * 128, :])
                lst[cb] = t
        wv_bf = []
        for cb in range(2):
            t = persist.tile([128, C], BF16, name=f"wvbf{cb}", tag=f"wvbf{cb}")
            nc.vector.tensor_copy(t, wv_sb[cb])
            wv_bf.append(t)

        def bias_tile(name, dt_):
            t = persist.tile([128, 2], F32, tag=name)
            nc.sync.dma_start(
                out=t, in_=bass.AP(tensor=dt_.ap().tensor, offset=0,
                                   ap=[[1, 128], [128, 2]]))
            return t

        bq_sb = bias_tile("bq", bq)
        bk_sb = bias_tile("bk", bk)
        bo_sb = bias_tile("bo", bo)
        bv_sb = bias_tile("bv", bv)
        ident = persist.tile([128, 128], BF16, name="ident", tag="ident")
        make_identity(nc, ident)
        zrow = persist.tile([128, 98], F32, name="zrow", tag="zrow")
        nc.vector.memset(zrow, 0.0)

        def conv1x1(x2d, w_sb, b_sb, out_tiles, xs_pool, ps_pool):
            """x2d [256, 9216] fp32 -> out_tiles bf16 [2][128, 9216], + bias."""
            for ch in range(6):
                xt = []
                for cb in range(2):
                    t = xs_pool.tile([128, 1536], F32R, name=f"xs{cb}",
                                     tag=f"xs{cb}", bufs=2)
                    nc.sync.dma_start(
                        out=t, in_=x2d[cb * 128:(cb + 1) * 128,
                                       ch * 1536:(ch + 1) * 1536])
                    xt.append(t)
                for coutb in range(2):
                    for pb in range(3):
                        ps = ps_pool.tile([128, 512], F32, name="cps", tag="cps")
                        for cb in range(2):
                            nc.tensor.matmul(
                                ps, w_sb[cb][:, coutb * 128:(coutb + 1) * 128],
                                xt[cb][:, pb * 512:(pb + 1) * 512],
                                start=(cb == 0), stop=(cb == 1))
                        o = ch * 1536 + pb * 512
                        bias_copy_alt(out_tiles[coutb][:, o:o + 512], ps,
                                      b_sb[:, coutb:coutb + 1])

        # ---------------- phases Q + A: Q/K conv and scores ----------------
        # pool open order = reverse close order (LIFO):
        #   PT1 (lives to end) < PT0 (to end of PV0) < P (to end of
        #   transposes) < S (to end of softmax) < qw (to end of A)
        esPT1 = ExitStack()
        p_PT1 = esPT1.enter_context(tc.tile_pool(name="PT1", bufs=1))
        pt1_t = [p_PT1.tile([128, NQ[1]], BF16, name=f"pt1_{i}", tag=f"pt1_{i}")
                 for i in range(NKP[1] // 128)]
        esPT0 = ExitStack()
        p_PT0 = esPT0.enter_context(tc.tile_pool(name="PT0", bufs=1))
        pt0_t = [p_PT0.tile([128, NQ[0]], BF16, name=f"pt0_{i}", tag=f"pt0_{i}")
                 for i in range(NKP[0] // 128)]
        pt_t = [pt0_t, pt1_t]
        esP = ExitStack()
        p_P = esP.enter_context(tc.tile_pool(name="P", bufs=1))
        p_t = [[p_P.tile([128, NKP[b]], BF16, name=f"p{b}_{i}", tag=f"p{b}_{i}")
                for i in range(len(NQB[b]))] for b in range(2)]
        esQW = ExitStack()
        p_qw = esQW.enter_context(tc.tile_pool(name="qw", bufs=1))
        qw = [p_qw.tile([128, NCH[b] * NTF[b]], BF16, name=f"qw{b}", tag=f"qw{b}")
              for b in range(2)]
        p_run = esQW.enter_context(tc.tile_pool(name="run", bufs=1))
        run_mx = [[p_run.tile([128, 1], F32, name=f"mx{b}_{i}", tag=f"mx{b}_{i}")
                   for i in range(len(NQB[b]))] for b in range(2)]
        run_ls = [[p_run.tile([128, 1], F32, name=f"ls{b}_{i}", tag=f"ls{b}_{i}")
                   for i in range(len(NQB[b]))] for b in range(2)]
        # branch1 pad columns of P stay 0 through the online rescales
        for i in range(len(NQB[1])):
            for kf in range(T):
                nc.gpsimd.memset(
                    p_t[1][i][:, kf * 160 + 144:(kf + 1) * 160], 0.0)

        with tc.tile_pool(name="qcm", bufs=1) as p_qcm, \
             tc.tile_pool(name="qxs", bufs=1) as p_qxs, \
             tc.tile_pool(name="qps", bufs=2, space="PSUM") as p_qps:
            q_cm = [p_qcm.tile([128, PIX], BF16, name=f"qcm{cb}", tag=f"qcm{cb}")
                    for cb in range(2)]
            conv1x1(xf.ap(), wq_sb, bq_sb, q_cm, p_qxs, p_qps)
            for b in range(2):
                psz, ohb = PSZ[b], OHB[b]
                qv = q_cm[b].rearrange("p (oh hh ow ww) -> p oh hh ow ww",
                                       oh=ohb, hh=psz, ow=ohb, ww=psz)
                for ci in range(NCH[b]):
                    wy, wx = divmod(ci, psz)
                    dst = qw[b][:, ci * NTF[b]:(ci + 1) * NTF[b]].rearrange(
                        "p (a c) -> p a c", a=ohb)
                    copy_rr(dst, qv[:, :, wy, :, wx])

        p_stat = esQW.enter_context(tc.tile_pool(name="stat", bufs=4))
        with tc.tile_pool(name="kcm", bufs=1) as p_kcm, \
             tc.tile_pool(name="kxs", bufs=1) as p_kxs, \
             tc.tile_pool(name="kps", bufs=2, space="PSUM") as p_kps, \
             tc.tile_pool(name="sps0", bufs=3, space="PSUM") as p_sps0, \
             tc.tile_pool(name="sps1", bufs=2, space="PSUM") as p_sps1:
            for kf in range(T):
                k_cm = [p_kcm.tile([128, PIX], BF16, name=f"kcm{cb}",
                                   tag=f"kcm{cb}") for cb in range(2)]
                conv1x1(xv.ap()[kf], wk_sb, bk_sb, k_cm, p_kxs, p_kps)
                for b in range(2):
                    psz, ohb, ntf = PSZ[b], OHB[b], NTF[b]
                    kv = k_cm[b].rearrange(
                        "p (oh hh ow ww) -> p oh hh ow ww",
                        oh=ohb, hh=psz, ow=ohb, ww=psz)
                    nmk = 2 if b == 0 else 1
                    mkw = ntf // nmk              # 288 / 144
                    for nqi, (q0, nqsz) in enumerate(NQB[b]):
                        for mkh in range(nmk):
                            ps = (p_sps0 if b == 0 else p_sps1).tile(
                                [128, mkw], F32, name=f"sps{b}", tag=f"sps{b}")
                            oh0 = mkh * (ohb // nmk)
                            for ci in range(NCH[b]):
                                wy, wx = divmod(ci, psz)
                                rhs = kv[:, oh0:oh0 + ohb // nmk, wy, :, wx]
                                lhsT = qw[b][:, ci * ntf + q0:
                                             ci * ntf + q0 + nqsz]
                                nc.tensor.matmul(
                                    ps[:nqsz], lhsT, rhs,
                                    start=(ci == 0), stop=(ci == NCH[b] - 1))
                            # online softmax over key blocks
                            o = kf * NTFP[b] + mkh * mkw
                            pt = p_t[b][nqi]
                            mx, ls = run_mx[b][nqi], run_ls[b][nqi]
                            bm = p_stat.tile([128, 1], F32, name="bm",
                                             tag="bm")
                            nc.vector.reduce_max(out=bm[:nqsz],
                                                 in_=ps[:nqsz, :],
                                                 axis=mybir.AxisListType.X)
                            first = (kf == 0 and mkh == 0)
                            if first:
                                nc.vector.tensor_copy(mx[:nqsz], bm[:nqsz])
                                nmx = p_stat.tile([128, 1], F32, name="nmx",
                                                  tag="nmx")
                                nc.vector.tensor_scalar_mul(
                                    nmx[:nqsz], mx[:nqsz], -SC[b])
                                nc.scalar.activation(
                                    out=pt[:nqsz, o:o + mkw],
                                    in_=ps[:nqsz, :], func=Exp,
                                    bias=nmx[:nqsz], scale=SC[b],
                                    accum_out=ls[:nqsz])
                            else:
                                nmax = p_stat.tile([128, 1], F32,
                                                   name="nmax", tag="nmax")
                                nc.vector.tensor_max(nmax[:nqsz], mx[:nqsz],
                                                     bm[:nqsz])
                                nmx = p_stat.tile([128, 1], F32, name="nmx",
                                                  tag="nmx")
                                nc.vector.tensor_scalar_mul(
                                    nmx[:nqsz], nmax[:nqsz], -SC[b])
                                delta = p_stat.tile([128, 1], F32,
                                                    name="delta", tag="delta")
                                nc.scalar.activation(
                                    out=delta[:nqsz], in_=mx[:nqsz],
                                    func=Exp, bias=nmx[:nqsz], scale=SC[b])
                                # rescale previously written P columns
                                nc.vector.tensor_scalar_mul(
                                    pt[:nqsz, 0:o], pt[:nqsz, 0:o],
                                    delta[:nqsz])
                                pl = p_stat.tile([128, 1], F32, name="pl",
                                                 tag="pl")
                                nc.scalar.activation(
                                    out=pt[:nqsz, o:o + mkw],
                                    in_=ps[:nqsz, :], func=Exp,
                                    bias=nmx[:nqsz], scale=SC[b],
                                    accum_out=pl[:nqsz])
                                nc.vector.scalar_tensor_tensor(
                                    out=ls[:nqsz], in0=ls[:nqsz],
                                    scalar=delta[:nqsz], in1=pl[:nqsz],
                                    op0=mybir.AluOpType.mult,
                                    op1=mybir.AluOpType.add)
                                nc.vector.tensor_copy(mx[:nqsz], nmax[:nqsz])
        # final normalization of P
        if "SM" not in PHASES:
            esQW.close(); esP.close(); esPT0.close(); esPT1.close()
            return nc
        for b in range(2):
            for nqi, (q0, nqsz) in enumerate(NQB[b]):
                rs = p_stat.tile([128, 1], F32, name="rs", tag="rs")
                nc.vector.reciprocal(rs[:nqsz], run_ls[b][nqi][:nqsz])
                nc.vector.tensor_scalar_mul(
                    p_t[b][nqi][:nqsz, :], p_t[b][nqi][:nqsz, :], rs[:nqsz])
        esQW.close()

        # ---------------- P^T transposes for both branches ----------------
        with tc.tile_pool(name="ptps", bufs=2, space="PSUM") as p_ptps:
            for br in range(2):
                if f"C{br}" not in PHASES:
                    continue
                for ti in range(NKP[br] // 128):
                    for nqi, (q0, nqsz) in enumerate(NQB[br]):
                        tp = p_ptps.tile([128, 128], BF16, name="ptps",
                                         tag="ptps")
                        nc.tensor.transpose(
                            tp[:, :nqsz],
                            p_t[br][nqi][:nqsz, ti * 128:(ti + 1) * 128],
                            ident[:nqsz, :nqsz])
                        alt[0] ^= 1
                        if alt[0]:
                            nc.scalar.copy(pt_t[br][ti][:, q0:q0 + nqsz],
                                           tp[:, :nqsz])
                        else:
                            nc.vector.tensor_copy(
                                pt_t[br][ti][:, q0:q0 + nqsz], tp[:, :nqsz])
        esP.close()

        # ---------------- phase C: V build + PV, per branch ----------------
        att0_dram = dramp.tile([128, 98 * 98], F32R, name="att0d", tag="att0d")
        esAtt1 = ExitStack()
        att_sb = {}

        for br in range(2):
            if f"C{br}" not in PHASES:
                continue
            psz, ohb, ntf = PSZ[br], OHB[br], NTF[br]
            sub = _subpieces(br)
            ntiles = len(sub)
            if br == 1:
                # att1 outlives V1 (used directly by phase D) -> open first
                p_att1 = esAtt1.enter_context(tc.tile_pool(name="att1", bufs=1))
            esV = ExitStack()
            p_V = esV.enter_context(tc.tile_pool(name=f"V{br}", bufs=1))
            v_t = [p_V.tile([128, NCH[br] * 128], BF16, name=f"v{br}_{i}",
                            tag=f"v{br}_{i}") for i in range(ntiles)]
            for ti in range(ntiles):
                if _pad_rows(br, ti, sub[ti]):
                    nc.gpsimd.memset(v_t[ti][:, :], 0.0)

            # --- V conv: x gathered window-major (bf16), x stationary ---
            with tc.tile_pool(name=f"xw{br}", bufs=1) as p_xw, \
                 tc.tile_pool(name=f"xl{br}", bufs=1) as p_xl, \
                 tc.tile_pool(name=f"vps{br}", bufs=4, space="PSUM") as p_vps:
                for kf in range(T):
                    xw = [p_xw.tile([128, NCH[br] * NTF[br]], BF16,
                                    name=f"xw{cb}", tag=f"xw{cb}")
                          for cb in range(2)]
                    nql = 4
                    csz = PIX // nql
                    ohc = ohb // nql
                    tpc = NTF[br] // nql
                    for chq in range(nql):
                        for cb in range(2):
                            xt = p_xl.tile([128, csz], F32R, name=f"xl{cb}",
                                           tag=f"xl{cb}",
                                           bufs=2 if br == 0 else 1)
                            nc.sync.dma_start(
                                out=xt,
                                in_=xv.ap()[kf, cb * 128:(cb + 1) * 128,
                                            chq * csz:(chq + 1) * csz])
                            xtv = xt.rearrange(
                                "p (oh hh ow ww) -> p oh hh ow ww",
                                oh=ohc, hh=psz, ow=ohb, ww=psz)
                            for ci in range(NCH[br]):
                                wy, wx = divmod(ci, psz)
                                dst = xw[cb][:, ci * ntf + chq * tpc:
                                             ci * ntf + (chq + 1) * tpc
                                             ].rearrange("p (a c) -> p a c",
                                                         a=ohc)
                                copy_rr(dst, xtv[:, :, wy, :, wx])
                    for ti in range(ntiles):
                        for (pkf, f0, m, off) in sub[ti]:
                            if pkf != kf:
                                continue
                            for ci in range(NCH[br]):
                                ps = p_vps.tile([128, 128], F32,
                                                name=f"vps{ci % 2}",
                                                tag=f"vps{ci % 2}")
                                for cb in range(2):
                                    lhsT = xw[cb][:, ci * ntf + f0:
                                                  ci * ntf + f0 + m]
                                    nc.tensor.matmul(
                                        ps[off:off + m], lhsT,
                                        wv_bf[cb][:, br * 128:(br + 1) * 128],
                                        start=(cb == 0), stop=(cb == 1),
                                        tile_position=(0, off))
                                dst = v_t[ti][off:off + m,
                                              ci * 128:(ci + 1) * 128]
                                alt[0] ^= 1
                                if alt[0]:
                                    nc.scalar.copy(dst, ps[off:off + m, :])
                                else:
                                    nc.vector.tensor_copy(dst,
                                                          ps[off:off + m, :])

            # --- PV: y^T accumulated over all key tiles; write into att ---
            esA2 = ExitStack()
            if br == 0:
                p_att = esA2.enter_context(tc.tile_pool(name="att0", bufs=1))
            else:
                p_att = p_att1
            att = p_att.tile([128, 98 * 98], F32R, name=f"att{br}",
                             tag=f"att{br}")
            att_sb[br] = att
            attv = att.rearrange("p (h w) -> p h w", h=98)
            nc.scalar.copy(att[:, 0:98], zrow)
            nc.scalar.copy(att[:, 97 * 98:98 * 98], zrow)
            zcol = zrow[:, 0:96].rearrange("p (a c) -> p a c", a=96)
            nc.vector.tensor_copy(attv[:, 1:97, 0:1], zcol)
            nc.vector.tensor_copy(attv[:, 1:97, 97:98], zcol)
            wvw = attv[:, 1:97, 1:97].rearrange(
                "p (oh hh) (ow ww) -> p oh hh ow ww", hh=psz, ww=psz)
            nqh_n = 2 if br == 0 else 1
            nqw = NQ[br] // nqh_n
            ohq = ohb // nqh_n
            with tc.tile_pool(name=f"pvps{br}", bufs=2,
                              space="PSUM") as p_pvps:
                for ci in range(NCH[br]):
                    wy, wx = divmod(ci, psz)
                    for nqh in range(nqh_n):
                        ps = p_pvps.tile([128, nqw], F32, name="pvps",
                                         tag="pvps")
                        for ti in range(ntiles):
                            nc.tensor.matmul(
                                ps, v_t[ti][:, ci * 128:(ci + 1) * 128],
                                pt_t[br][ti][:, nqh * nqw:(nqh + 1) * nqw],
                                start=(ti == 0), stop=(ti == ntiles - 1))
                        dst = wvw[:, nqh * ohq:(nqh + 1) * ohq, wy, :, wx]
                        src = ps.rearrange("p (a c) -> p a c", a=ohq)
                        bias_copy_alt(dst, src, bv_sb[:, br:br + 1])
            if br == 0:
                nc.sync.dma_start(out=att0_dram, in_=att)
                esA2.close()
            esV.close()
            if br == 0:
                esPT0.close()

        # ---------------- phase D: 3x3 conv + LeakyReLU ----------------
        if "D" not in PHASES:
            esAtt1.close(); esPT1.close()
            return nc
        with tc.tile_pool(name="attr", bufs=1) as p_attr, \
             tc.tile_pool(name="wot", bufs=1) as p_wot, \
             tc.tile_pool(name="dout", bufs=3) as p_do, \
             tc.tile_pool(name="dps", bufs=4, space="PSUM") as p_dps:
            att0 = p_attr.tile([128, 98 * 98], F32R, name="attr0", tag="attr0")
            nc.sync.dma_start(out=att0, in_=att0_dram)
            att_in = [att0, att_sb[1]]
            wot_sb = []
            for cb in range(2):
                t = p_wot.tile([128, 9, C], F32R, name=f"wot{cb}",
                               tag=f"wot{cb}")
                nc.sync.dma_start(
                    out=t,
                    in_=wot.ap()[:, cb * 128:(cb + 1) * 128, :].rearrange(
                        "t i o -> i t o"))
                wot_sb.append(t)
            attv2 = [att_in[cb].rearrange("p (h w) -> p h w", h=98)
                     for cb in range(2)]
            for coutb in range(2):
                for rg in range(24):
                    ps = p_dps.tile([128, 384], F32, name="dps", tag="dps")
                    k = 0
                    for cb in range(2):
                        for tap in range(9):
                            dy, dx = divmod(tap, 3)
                            rhs = attv2[cb][:, rg * 4 + dy:rg * 4 + dy + 4,
                                            dx:dx + 96]
                            lhsT = wot_sb[cb][:, tap,
                                              coutb * 128:(coutb + 1) * 128]
                            nc.tensor.matmul(ps, lhsT, rhs,
                                             start=(k == 0), stop=(k == 17))
                            k += 1
                    t1 = p_do.tile([128, 384], F32, name="t1", tag="t1")
                    nc.scalar.activation(out=t1, in_=ps, func=Identity,
                                         bias=bo_sb[:, coutb:coutb + 1],
                                         scale=1.0)
                    t2 = p_do.tile([128, 384], F32, name="t2", tag="t2")
                    nc.vector.scalar_tensor_tensor(
                        out=t2, in0=t1, scalar=0.2, in1=t1,
                        op0=mybir.AluOpType.mult, op1=mybir.AluOpType.max)
                    nc.sync.dma_start(
                        out=out.ap()[coutb * 128:(coutb + 1) * 128,
                                     rg * 384:(rg + 1) * 384],
                        in_=t2)
        esAtt1.close()
        esPT1.close()
    return nc


_CACHED = {}


def _get_nc():
    if "nc" not in _CACHED:
        nc = bacc.Bacc("TRN2", debug=False, target_bir_lowering=False)
        build(nc)
        nc.compile()
        _CACHED["nc"] = nc
    return _CACHED["nc"]


def make_in_maps(x, wq, bq_, wk, bk_, wv, bv_, wo, bo_):
    shared = {
        "wqt": np.ascontiguousarray(wq.T.astype(np.float32)),
        "wkt": np.ascontiguousarray(wk.T.astype(np.float32)),
        "wvt": np.ascontiguousarray(wv.T.astype(np.float32)),
        "wot": np.ascontiguousarray(
            wo.transpose(2, 3, 1, 0).reshape(9, C, C).astype(np.float32)),
        "bq": np.ascontiguousarray(bq_.astype(np.float32)),
        "bk": np.ascontiguousarray(bk_.astype(np.float32)),
        "bv": np.ascontiguousarray(bv_.astype(np.float32)),
        "bo": np.ascontiguousarray(bo_.astype(np.float32)),
    }
    x3 = np.ascontiguousarray(x.reshape(2 * T, C, PIX).astype(np.float32))
    in_maps = []
    for core in range(NCORES):
        v, f = divmod(core, T)
        m = dict(shared)
        m["xv"] = np.ascontiguousarray(x3[v * T:(v + 1) * T])
        m["xf"] = np.ascontiguousarray(x3[v * T + f])
        in_maps.append(m)
    return in_maps


def kernel(**inputs):
    from concourse.bass_utils import run_bass_kernel_spmd

    x = np.asarray(inputs["x"], dtype=np.float32)
    in_maps = make_in_maps(
        x, np.asarray(inputs["wq"]), np.asarray(inputs["bq"]),
        np.asarray(inputs["wk"]), np.asarray(inputs["bk"]),
        np.asarray(inputs["wv"]), np.asarray(inputs["bv"]),
        np.asarray(inputs["wo"]), np.asarray(inputs["bo"]))
    nc = _get_nc()
    res = run_bass_kernel_spmd(nc, in_maps, core_ids=list(range(NCORES)))
    outs = [res.results[c]["out"].reshape(C, H, W) for c in range(NCORES)]
    return np.stack(outs).astype(np.float32)



# revision 10
# speedup vs baseline: 1.2876x; 1.2876x over previous
"""Trainium2 Bass kernel for nn_MultiHeadedAttention_6416681140387.

Two-branch windowed video attention:
  x [8,256,96,96] -> 1x1 conv Q/K/V -> per-branch full attention over
  window-token features (branch0: 4x4 patches, d=2048, 2304 key tokens;
  branch1: 8x8 patches, d=8192, 576 key tokens) -> concat channels
  -> 3x3 conv + LeakyReLU(0.2).

Sharding: 8 cores = (video b in {0,1}) x (frame t in {0..3}). Each core
computes its full output frame [256,96,96]; K/V are recomputed per core
from its 4-frame video slice (no collectives). Host rotates frames so
xv[0] is the core's own frame; P columns and V tokens both use processed
order, so attention math is order-invariant.

All matmuls are bf16 with fp32 PSUM accumulation. x loads via SWDGE
cast-DMA (f32 DRAM -> bf16 SBUF); a bf16 copy of x is stashed to DRAM
during the K loop and re-read by the two V passes. Window gathers are a
handful of big multi-dim strided copies (Q's gather is folded into the
conv PSUM evacuation). V^T tiles are frame-aligned: br0 = 5 tiles/frame
(last 64 tokens short), br1 = 1 full tile/frame + one shared spill tile
(16 tokens/frame at partition offset f*32). Attention outputs stay in
SBUF (bf16) through the 3x3 conv.
"""

import sys

if "/opt/trn_rl_repo" not in sys.path:
    sys.path.insert(0, "/opt/trn_rl_repo")

import math
from contextlib import ExitStack

import numpy as np

import concourse.bass as bass
import concourse.tile as tile
from concourse import bacc, mybir
from concourse.masks import make_identity

F32 = mybir.dt.float32
F32R = mybir.dt.float32r
BF16 = mybir.dt.bfloat16

T = 4
C = 256
H = W = 96
PIX = H * W
NCORES = 8

PSZ = [4, 8]
OHB = [24, 12]                  # token grid side per branch
NTF = [576, 144]                # real tokens per frame
NTFP = [640, 144]               # P-column stride per frame
NKP = [2560, 640]               # key-token tiles * 128 per video
NQ = [576, 144]                 # query tokens (one frame)
NCH = [16, 64]                  # d-chunks (psz^2)
SC = [1.0 / math.sqrt(2048.0), 1.0 / math.sqrt(8192.0)]
NQB = [[(0, 128), (128, 128), (256, 128), (384, 128), (512, 64)],
       [(0, 128), (128, 16)]]

Exp = mybir.ActivationFunctionType.Exp
Identity = mybir.ActivationFunctionType.Identity


def build(nc):
    xv = nc.dram_tensor("xv", [T, C, PIX], F32R, kind="ExternalInput")
    wqt = nc.dram_tensor("wqt", [C, C], F32R, kind="ExternalInput")
    wkt = nc.dram_tensor("wkt", [C, C], F32R, kind="ExternalInput")
    wvt = nc.dram_tensor("wvt", [C, C], F32R, kind="ExternalInput")
    wot = nc.dram_tensor("wot", [9, C, C], F32R, kind="ExternalInput")
    bq = nc.dram_tensor("bq", [C], F32, kind="ExternalInput")
    bk = nc.dram_tensor("bk", [C], F32, kind="ExternalInput")
    bv = nc.dram_tensor("bv", [C], F32, kind="ExternalInput")
    bo = nc.dram_tensor("bo", [C], F32, kind="ExternalInput")
    out = nc.dram_tensor("out", [C, PIX], F32, kind="ExternalOutput")

    alt = [0]

    def evac_alt(dst, src, bias_ap=None):
        """PSUM -> SBUF evacuation, alternating scalar/vector engines."""
        alt[0] ^= 1
        if bias_ap is not None:
            if alt[0]:
                nc.scalar.activation(out=dst, in_=src, func=Identity,
                                     bias=bias_ap, scale=1.0)
            else:
                nc.vector.tensor_scalar_add(dst, src, bias_ap)
        else:
            if alt[0]:
                nc.scalar.copy(dst, src)
            else:
                nc.vector.tensor_copy(dst, src)

    galt = [0]

    def gather_alt(dst, src):
        galt[0] ^= 1
        if galt[0]:
            nc.vector.tensor_copy(dst, src)
        else:
            nc.scalar.copy(dst, src)

    with tile.TileContext(nc, pool_alloc_mode="queue") as tc, ExitStack() as top:
        persist = top.enter_context(tc.tile_pool(name="persist", bufs=1))
        dramp = top.enter_context(tc.tile_pool(name="dram", bufs=1,
                                               space="DRAM"))

        # bf16 weights via cast-DMA
        wq_sb, wk_sb, wv_sb = [None, None], [None, None], [None, None]
        for name, dt_, lst in (("wq", wqt, wq_sb), ("wk", wkt, wk_sb),
                               ("wv", wvt, wv_sb)):
            for cb in range(2):
                t = persist.tile([128, C], BF16, name=f"{name}{cb}",
                                 tag=f"{name}{cb}")
                nc.gpsimd.dma_start(out=t,
                                    in_=dt_.ap()[cb * 128:(cb + 1) * 128, :])
                lst[cb] = t

        def bias_tile(name, dt_):
            t = persist.tile([128, 2], F32, tag=name)
            nc.sync.dma_start(
                out=t, in_=bass.AP(tensor=dt_.ap().tensor, offset=0,
                                   ap=[[1, 128], [128, 2]]))
            return t

        bq_sb = bias_tile("bq", bq)
        bk_sb = bias_tile("bk", bk)
        bv_sb = bias_tile("bv", bv)
        bo_sb = bias_tile("bo", bo)
        ident = persist.tile([128, 128], BF16, name="ident", tag="ident")
        make_identity(nc, ident)
        zrow = persist.tile([128, 98], BF16, name="zrow", tag="zrow")
        nc.vector.memset(zrow, 0.0)

        # bf16 stash of x in DRAM, written during the K loop
        xbf_d = [[dramp.tile([128, PIX], BF16, name=f"xd{j}{cb}",
                             tag=f"xd{j}{cb}") for cb in range(2)]
                 for j in range(T)]

        # Pool open order = reverse close order (LIFO):
        #   att (lives to end of D) < PT1 (to end of PV1) < PT0 (to end
        #   of PV0) < P (to end of transposes) < qw (to end of scores).
        # Tiles are created lazily at first use.
        esAtt = ExitStack()
        p_att = esAtt.enter_context(tc.tile_pool(name="att", bufs=1))
        esPT1 = ExitStack()
        p_PT1 = esPT1.enter_context(tc.tile_pool(name="PT1", bufs=1))
        esPT0 = ExitStack()
        p_PT0 = esPT0.enter_context(tc.tile_pool(name="PT0", bufs=1))

        # ---------------- phase A: per-frame QK conv + scores ------------
        esP = ExitStack()
        p_P = esP.enter_context(tc.tile_pool(name="P", bufs=1))
        p_t = [[p_P.tile([128, NKP[b]], BF16, name=f"p{b}_{i}",
                         tag=f"p{b}_{i}")
                for i in range(len(NQB[b]))] for b in range(2)]
        for b in range(2):
            for i in range(len(NQB[b])):
                nc.gpsimd.memset(p_t[b][i][:, :], 0.0)

        esQW = ExitStack()
        p_qw = esQW.enter_context(tc.tile_pool(name="qw", bufs=1))
        qw = [p_qw.tile([128, NCH[b] * NTF[b]], BF16, name=f"qw{b}",
                        tag=f"qw{b}") for b in range(2)]
        p_run = esQW.enter_context(tc.tile_pool(name="run", bufs=1))
        run_mx = [[p_run.tile([128, 1], F32, name=f"mx{b}_{i}",
                              tag=f"mx{b}_{i}")
                   for i in range(len(NQB[b]))] for b in range(2)]
        run_ls = [[p_run.tile([128, 1], F32, name=f"ls{b}_{i}",
                              tag=f"ls{b}_{i}")
                   for i in range(len(NQB[b]))] for b in range(2)]
        p_stat = esQW.enter_context(tc.tile_pool(name="stat", bufs=4))

        ext = [[0 for _ in NQB[b]] for b in range(2)]   # rescale extent

        def conv_k(x_tiles, out_tiles, ps_pool):
            """K conv: bf16 x [2][128,9216] -> k_cm bf16 (+bias)."""
            for coutb in range(2):
                for pg in range(9):
                    ps = ps_pool.tile([128, 1024], F32, name="cps",
                                      tag="cps")
                    for half in range(2):
                        o = pg * 1024 + half * 512
                        for cb in range(2):
                            nc.tensor.matmul(
                                ps[:, half * 512:(half + 1) * 512],
                                wk_sb[cb][:, coutb * 128:(coutb + 1) * 128],
                                x_tiles[cb][:, o:o + 512],
                                start=(cb == 0), stop=(cb == 1))
                    evac_alt(out_tiles[coutb][:, pg * 1024:(pg + 1) * 1024],
                             ps, bk_sb[:, coutb:coutb + 1])

        def conv_q(x_tiles, ps_pool):
            """Q conv with PSUM evacuated straight into token-major qw."""
            # branch0 (coutb 0): one psum region per token row (384 pix)
            d0 = qw[0].rearrange("p (wy wx oh ow) -> p wy wx oh ow",
                                 wy=4, wx=4, oh=24)
            for g in range(12):               # 2 token rows per psum
                ps = ps_pool.tile([128, 1024], F32, name="cps", tag="cps")
                for half in range(2):
                    oh = g * 2 + half
                    for cb in range(2):
                        nc.tensor.matmul(
                            ps[:, half * 512:half * 512 + 384],
                            wq_sb[cb][:, 0:128],
                            x_tiles[cb][:, oh * 384:(oh + 1) * 384],
                            start=(cb == 0), stop=(cb == 1))
                for half in range(2):
                    oh = g * 2 + half
                    src = ps[:, half * 512:half * 512 + 384].rearrange(
                        "p (wy ow wx) -> p wy wx ow", wy=4, ow=24)
                    evac_alt(d0[:, :, :, oh], src, bq_sb[:, 0:1])
            # branch1 (coutb 1): half a token row (4 of 8 wy) per region
            d1 = qw[1].rearrange("p (wy wx oh ow) -> p wy wx oh ow",
                                 wy=8, wx=8, oh=12)
            for g in range(12):
                ps = ps_pool.tile([128, 1024], F32, name="cps", tag="cps")
                for half in range(2):
                    o = g * 768 + half * 384
                    for cb in range(2):
                        nc.tensor.matmul(
                            ps[:, half * 512:half * 512 + 384],
                            wq_sb[cb][:, 128:256],
                            x_tiles[cb][:, o:o + 384],
                            start=(cb == 0), stop=(cb == 1))
                for half in range(2):
                    oh, wyh = divmod(g * 2 + half, 2)
                    src = ps[:, half * 512:half * 512 + 384].rearrange(
                        "p (wy ow wx) -> p wy wx ow", wy=4, ow=12)
                    evac_alt(d1[:, wyh * 4:(wyh + 1) * 4, :, oh], src,
                             bq_sb[:, 1:2])

        with tc.tile_pool(name="kx", bufs=1) as p_kx, \
             tc.tile_pool(name="kcm", bufs=1) as p_kcm, \
             tc.tile_pool(name="kps", bufs=2, space="PSUM") as p_kps, \
             tc.tile_pool(name="sps0", bufs=2, space="PSUM") as p_sps0, \
             tc.tile_pool(name="sps1", bufs=2, space="PSUM") as p_sps1:
            for j in range(T):
                xb = []
                for cb in range(2):
                    t = p_kx.tile([128, PIX], BF16, name=f"kx{cb}",
                                  tag=f"kx{cb}")
                    nc.gpsimd.dma_start(
                        out=t, in_=xv.ap()[j, cb * 128:(cb + 1) * 128, :])
                    nc.sync.dma_start(out=xbf_d[j][cb], in_=t)
                    xb.append(t)
                k_cm = [p_kcm.tile([128, PIX], BF16, name=f"kcm{cb}",
                                   tag=f"kcm{cb}") for cb in range(2)]
                conv_k(xb, k_cm, p_kps)
                if j == 0:
                    conv_q(xb, p_kps)

                # ---- scores for key frame j, both branches ----
                for b in range(2):
                    psz, ohb, ntf = PSZ[b], OHB[b], NTF[b]
                    kv = k_cm[b].rearrange(
                        "p (oh hh ow ww) -> p oh hh ow ww",
                        oh=ohb, hh=psz, ow=ohb, ww=psz)
                    nmk = 2 if b == 0 else 1
                    mkw = ntf // nmk              # 288 / 144
                    for nqi, (q0, nqsz) in enumerate(NQB[b]):
                        for mkh in range(nmk):
                            ps = (p_sps0 if b == 0 else p_sps1).tile(
                                [128, mkw], F32, name=f"sps{b}",
                                tag=f"sps{b}")
                            oh0 = mkh * (ohb // nmk)
                            for ci in range(NCH[b]):
                                wy, wx = divmod(ci, psz)
                                rhs = kv[:, oh0:oh0 + ohb // nmk, wy, :, wx]
                                lhsT = qw[b][:, ci * ntf + q0:
                                             ci * ntf + q0 + nqsz]
                                nc.tensor.matmul(
                                    ps[:nqsz], lhsT, rhs,
                                    start=(ci == 0),
                                    stop=(ci == NCH[b] - 1))
                            # online softmax over key blocks
                            o = j * NTFP[b] + mkh * mkw
                            pt = p_t[b][nqi]
                            mx, ls = run_mx[b][nqi], run_ls[b][nqi]
                            bm = p_stat.tile([128, 1], F32, name="bm",
                                             tag="bm")
                            nc.vector.reduce_max(out=bm[:nqsz],
                                                 in_=ps[:nqsz, :],
                                                 axis=mybir.AxisListType.X)
                            if j == 0 and mkh == 0:
                                nc.vector.tensor_copy(mx[:nqsz], bm[:nqsz])
                                nmx = p_stat.tile([128, 1], F32, name="nmx",
                                                  tag="nmx")
                                nc.vector.tensor_scalar_mul(
                                    nmx[:nqsz], mx[:nqsz], -SC[b])
                                nc.scalar.activation(
                                    out=pt[:nqsz, o:o + mkw],
                                    in_=ps[:nqsz, :], func=Exp,
                                    bias=nmx[:nqsz], scale=SC[b],
                                    accum_out=ls[:nqsz])
                            else:
                                nmax = p_stat.tile([128, 1], F32,
                                                   name="nmax", tag="nmax")
                                nc.vector.tensor_max(nmax[:nqsz], mx[:nqsz],
                                                     bm[:nqsz])
                                nmx = p_stat.tile([128, 1], F32, name="nmx",
                                                  tag="nmx")
                                nc.vector.tensor_scalar_mul(
                                    nmx[:nqsz], nmax[:nqsz], -SC[b])
                                delta = p_stat.tile([128, 1], F32,
                                                    name="delta",
                                                    tag="delta")
                                nc.scalar.activation(
                                    out=delta[:nqsz], in_=mx[:nqsz],
                                    func=Exp, bias=nmx[:nqsz], scale=SC[b])
                                e = ext[b][nqi]
                                nc.vector.tensor_scalar_mul(
                                    pt[:nqsz, 0:e], pt[:nqsz, 0:e],
                                    delta[:nqsz])
                                pl = p_stat.tile([128, 1], F32, name="pl",
                                                 tag="pl")
                                nc.scalar.activation(
                                    out=pt[:nqsz, o:o + mkw],
                                    in_=ps[:nqsz, :], func=Exp,
                                    bias=nmx[:nqsz], scale=SC[b],
                                    accum_out=pl[:nqsz])
                                nc.vector.scalar_tensor_tensor(
                                    out=ls[:nqsz], in0=ls[:nqsz],
                                    scalar=delta[:nqsz], in1=pl[:nqsz],
                                    op0=mybir.AluOpType.mult,
                                    op1=mybir.AluOpType.add)
                                nc.vector.tensor_copy(mx[:nqsz],
                                                      nmax[:nqsz])
                            ext[b][nqi] = max(ext[b][nqi], o + mkw)

        # final normalization of P
        for b in range(2):
            for nqi, (q0, nqsz) in enumerate(NQB[b]):
                rs = p_stat.tile([128, 1], F32, name="rs", tag="rs")
                nc.vector.reciprocal(rs[:nqsz], run_ls[b][nqi][:nqsz])
                nc.vector.tensor_scalar_mul(
                    p_t[b][nqi][:nqsz, :], p_t[b][nqi][:nqsz, :], rs[:nqsz])
        esQW.close()

        # ---------------- P^T transposes for both branches ----------------
        pt1_t = [p_PT1.tile([128, NQ[1]], BF16, name=f"pt1_{i}",
                            tag=f"pt1_{i}") for i in range(5)]
        nc.gpsimd.memset(pt1_t[4][:, :], 0.0)
        pt0_t = [p_PT0.tile([128, NQ[0]], BF16, name=f"pt0_{i}",
                            tag=f"pt0_{i}") for i in range(NKP[0] // 128)]

        with tc.tile_pool(name="ptps", bufs=4, space="PSUM") as p_ptps:
            for ti in range(NKP[0] // 128):
                for nqi, (q0, nqsz) in enumerate(NQB[0]):
                    tp = p_ptps.tile([128, 128], BF16, name="ptps",
                                     tag="ptps")
                    nc.tensor.transpose(
                        tp[:, :nqsz],
                        p_t[0][nqi][:nqsz, ti * 128:(ti + 1) * 128],
                        ident[:nqsz, :nqsz])
                    evac_alt(pt0_t[ti][:, q0:q0 + nqsz], tp[:, :nqsz])
            for f in range(T):
                for nqi, (q0, nqsz) in enumerate(NQB[1]):
                    tp = p_ptps.tile([128, 128], BF16, name="ptps",
                                     tag="ptps")
                    nc.tensor.transpose(
                        tp[:, :nqsz],
                        p_t[1][nqi][:nqsz, f * 144:f * 144 + 128],
                        ident[:nqsz, :nqsz])
                    evac_alt(pt1_t[f][:, q0:q0 + nqsz], tp[:, :nqsz])
                    tp2 = p_ptps.tile([128, 128], BF16, name="ptps2",
                                      tag="ptps2")
                    nc.tensor.transpose(
                        tp2[:16, :nqsz],
                        p_t[1][nqi][:nqsz, f * 144 + 128:(f + 1) * 144],
                        ident[:nqsz, :nqsz])
                    evac_alt(pt1_t[4][f * 32:f * 32 + 16, q0:q0 + nqsz],
                             tp2[:16, :nqsz])
        esP.close()

        # ---------------- att tiles (SBUF-resident, bf16) -----------------
        att_sb = {}

        def init_att(br):
            att = p_att.tile([128, 98 * 98], BF16, name=f"att{br}",
                             tag=f"att{br}")
            att_sb[br] = att
            attv = att.rearrange("p (h w) -> p h w", h=98)
            nc.gpsimd.tensor_copy(att[:, 0:98], zrow)
            nc.gpsimd.tensor_copy(att[:, 97 * 98:98 * 98], zrow)
            zcol = zrow[:, 0:96].rearrange("p (a c) -> p a c", a=96)
            nc.gpsimd.tensor_copy(attv[:, 1:97, 0:1], zcol)
            nc.gpsimd.tensor_copy(attv[:, 1:97, 97:98], zcol)
            return attv

        # ---------------- phase C: V build + PV, per branch ----------------
        for br in range(2):
            psz, ohb, ntf = PSZ[br], OHB[br], NTF[br]
            ntiles = NKP[br] // 128
            esV = ExitStack()
            p_V = esV.enter_context(tc.tile_pool(name=f"V{br}", bufs=1))
            v_t = [p_V.tile([128, NCH[br] * 128], BF16, name=f"v{br}_{i}",
                            tag=f"v{br}_{i}") for i in range(ntiles)]
            if br == 0:
                for f in range(T):
                    nc.gpsimd.memset(v_t[5 * f + 4][64:128, :], 0.0)
            else:
                nc.gpsimd.memset(v_t[4][:, :], 0.0)

            with tc.tile_pool(name=f"vx{br}", bufs=2) as p_vx, \
                 tc.tile_pool(name=f"xw{br}", bufs=1) as p_xw, \
                 tc.tile_pool(name=f"vps{br}", bufs=2,
                              space="PSUM") as p_vps:
                nquart = 4
                csz = PIX // nquart
                ohc = ohb // nquart
                for j in range(T):
                    xw = [p_xw.tile([128, NCH[br] * ntf], BF16,
                                    name=f"xw{cb}", tag=f"xw{cb}")
                          for cb in range(2)]
                    d5 = [xw[cb].rearrange(
                        "p (wy wx oh ow) -> p wy wx oh ow",
                        wy=psz, wx=psz, oh=ohb) for cb in range(2)]
                    for q in range(nquart):
                        for cb in range(2):
                            xt = p_vx.tile([128, csz], BF16,
                                           name=f"vx{cb}", tag=f"vx{cb}")
                            nc.sync.dma_start(
                                out=xt,
                                in_=xbf_d[j][cb][:, q * csz:(q + 1) * csz])
                            s5 = xt.rearrange(
                                "p (oh wy ow wx) -> p wy wx oh ow",
                                oh=ohc, wy=psz, ow=ohb)
                            for wy in range(psz):
                                gather_alt(
                                    d5[cb][:, wy, :,
                                           q * ohc:(q + 1) * ohc],
                                    s5[:, wy])
                    # V matmuls for frame j
                    if br == 0:
                        for sub in range(5):
                            m = 128 if sub < 4 else 64
                            t0 = sub * 128
                            ti = 5 * j + sub
                            ps = p_vps.tile([128, 2048], F32, name="vps",
                                            tag="vps")
                            for ci in range(16):
                                for cb in range(2):
                                    lhsT = xw[cb][:, ci * ntf + t0:
                                                  ci * ntf + t0 + m]
                                    nc.tensor.matmul(
                                        ps[:m, ci * 128:(ci + 1) * 128],
                                        lhsT,
                                        wv_sb[cb][:, 0:128],
                                        start=(cb == 0),
                                        stop=(cb == 1))
                            evac_alt(v_t[ti][:m, :], ps[:m, :])
                    else:
                        for cg in range(4):
                            ps = p_vps.tile([128, 2048], F32, name="vps",
                                            tag="vps")
                            for cl in range(16):
                                ci = cg * 16 + cl
                                for cb in range(2):
                                    lhsT = xw[cb][:, ci * ntf:
                                                  ci * ntf + 128]
                                    nc.tensor.matmul(
                                        ps[:, cl * 128:(cl + 1) * 128],
                                        lhsT,
                                        wv_sb[cb][:, 128:256],
                                        start=(cb == 0),
                                        stop=(cb == 1))
                            evac_alt(v_t[j][:, cg * 2048:(cg + 1) * 2048],
                                     ps)
                        off = j * 32
                        for cg in range(4):
                            ps = p_vps.tile([128, 2048], F32, name="vps",
                                            tag="vps")
                            for cl in range(16):
                                ci = cg * 16 + cl
                                for cb in range(2):
                                    lhsT = xw[cb][:, ci * ntf + 128:
                                                  ci * ntf + 144]
                                    nc.tensor.matmul(
                                        ps[off:off + 16,
                                           cl * 128:(cl + 1) * 128],
                                        lhsT,
                                        wv_sb[cb][:, 128:256],
                                        start=(cb == 0),
                                        stop=(cb == 1),
                                        tile_position=(0, off))
                            evac_alt(
                                v_t[4][off:off + 16,
                                       cg * 2048:(cg + 1) * 2048],
                                ps[off:off + 16, :])

            # --- PV: y^T accumulated over all key tiles; write into att ---
            attv = init_att(br)
            wvw = attv[:, 1:97, 1:97].rearrange(
                "p (oh hh) (ow ww) -> p oh hh ow ww", hh=psz, ww=psz)
            pt_t = pt0_t if br == 0 else pt1_t
            nqh_n = 2 if br == 0 else 1
            nqw = NQ[br] // nqh_n
            ohq = ohb // nqh_n
            with tc.tile_pool(name=f"pvps{br}", bufs=4,
                              space="PSUM") as p_pvps:
                for ci in range(NCH[br]):
                    wy, wx = divmod(ci, psz)
                    for nqh in range(nqh_n):
                        ps = p_pvps.tile([128, nqw], F32, name="pvps",
                                         tag="pvps")
                        for ti in range(ntiles):
                            nc.tensor.matmul(
                                ps, v_t[ti][:, ci * 128:(ci + 1) * 128],
                                pt_t[ti][:, nqh * nqw:(nqh + 1) * nqw],
                                start=(ti == 0), stop=(ti == ntiles - 1))
                        dst = wvw[:, nqh * ohq:(nqh + 1) * ohq, wy, :, wx]
                        src = ps.rearrange("p (a c) -> p a c", a=ohq)
                        evac_alt(dst, src, bv_sb[:, br:br + 1])
            esV.close()
            if br == 0:
                esPT0.close()
        esPT1.close()

        # ---------------- phase D: 3x3 conv + LeakyReLU ----------------
        with tc.tile_pool(name="wot", bufs=1) as p_wot, \
             tc.tile_pool(name="dout", bufs=3) as p_do, \
             tc.tile_pool(name="dps", bufs=2, space="PSUM") as p_dps:
            wot_sb = []
            for cb in range(2):
                t = p_wot.tile([128, 9, C], BF16, name=f"wot{cb}",
                               tag=f"wot{cb}")
                nc.gpsimd.dma_start(
                    out=t,
                    in_=wot.ap()[:, cb * 128:(cb + 1) * 128, :].rearrange(
                        "t i o -> i t o"))
                wot_sb.append(t)
            attv2 = [att_sb[cb].rearrange("p (h w) -> p h w", h=98)
                     for cb in range(2)]
            for coutb in range(2):
                for g in range(6):            # groups of 4 row-groups
                    ps = p_dps.tile([128, 2048], F32, name="dps",
                                    tag="dps")
                    for cb in range(2):
                        for tap in range(9):
                            dy, dx = divmod(tap, 3)
                            lhsT = wot_sb[cb][:, tap,
                                              coutb * 128:(coutb + 1) * 128]
                            for rg_ in range(4):
                                rg = g * 4 + rg_
                                rhs = attv2[cb][:, rg * 4 + dy:
                                                rg * 4 + dy + 4,
                                                dx:dx + 96]
                                nc.tensor.matmul(
                                    ps[:, rg_ * 512:rg_ * 512 + 384],
                                    lhsT, rhs,
                                    start=(cb == 0 and tap == 0),
                                    stop=(cb == 1 and tap == 8))
                    psv = ps.rearrange("p (a c) -> p a c", a=4)[:, :, 0:384]
                    t1 = p_do.tile([128, 1536], F32, name="t1", tag="t1")
                    t1v = t1.rearrange("p (a c) -> p a c", a=4)
                    nc.scalar.activation(out=t1v, in_=psv, func=Identity,
                                         bias=bo_sb[:, coutb:coutb + 1],
                                         scale=1.0)
                    t2 = p_do.tile([128, 1536], F32, name="t2", tag="t2")
                    nc.vector.scalar_tensor_tensor(
                        out=t2, in0=t1, scalar=0.2, in1=t1,
                        op0=mybir.AluOpType.mult,
                        op1=mybir.AluOpType.max)
                    nc.sync.dma_start(
                        out=out.ap()[coutb * 128:(coutb + 1) * 128,
                                     g * 1536:(g + 1) * 1536],
                        in_=t2)
        esAtt.close()
    return nc


_CACHED = {}


def _get_nc():
    if "nc" not in _CACHED:
        nc = bacc.Bacc("TRN2", debug=False, target_bir_lowering=False)
        build(nc)
        nc.compile()
        _CACHED["nc"] = nc
    return _CACHED["nc"]


def make_in_maps(x, wq, bq_, wk, bk_, wv, bv_, wo, bo_):
    shared = {
        "wqt": np.ascontiguousarray(wq.T.astype(np.float32)),
        "wkt": np.ascontiguousarray(wk.T.astype(np.float32)),
        "wvt": np.ascontiguousarray(wv.T.astype(np.float32)),
        "wot": np.ascontiguousarray(
            wo.transpose(2, 3, 1, 0).reshape(9, C, C).astype(np.float32)),
        "bq": np.ascontiguousarray(bq_.astype(np.float32)),
        "bk": np.ascontiguousarray(bk_.astype(np.float32)),
        "bv": np.ascontiguousarray(bv_.astype(np.float32)),
        "bo": np.ascontiguousarray(bo_.astype(np.float32)),
    }
    x3 = np.ascontiguousarray(x.reshape(2 * T, C, PIX).astype(np.float32))
    in_maps = []
    for core in range(NCORES):
        v, f = divmod(core, T)
        # rotate so the core's own frame is xv[0]; P and V both use
        # processed order, so attention math is order-invariant.
        order = [f] + [k for k in range(T) if k != f]
        m = dict(shared)
        m["xv"] = np.ascontiguousarray(x3[[v * T + k for k in order]])
        in_maps.append(m)
    return in_maps


def kernel(**inputs):
    from concourse.bass_utils import run_bass_kernel_spmd

    x = np.asarray(inputs["x"], dtype=np.float32)
    in_maps = make_in_maps(
        x, np.asarray(inputs["wq"]), np.asarray(inputs["bq"]),
        np.asarray(inputs["wk"]), np.asarray(inputs["bk"]),
        np.asarray(inputs["wv"]), np.asarray(inputs["bv"]),
        np.asarray(inputs["wo"]), np.asarray(inputs["bo"]))
    nc = _get_nc()
    res = run_bass_kernel_spmd(nc, in_maps, core_ids=list(range(NCORES)))
    outs = [res.results[c]["out"].reshape(C, H, W) for c in range(NCORES)]
    return np.stack(outs).astype(np.float32)


# revision 23
# speedup vs baseline: 1.4621x; 1.1355x over previous
"""Trainium2 Bass kernel for nn_MultiHeadedAttention_6416681140387.

Two-branch windowed video attention:
  x [8,256,96,96] -> 1x1 conv Q/K/V -> per-branch full attention over
  window-token features (branch0: 4x4 patches, d=2048, 2304 key tokens;
  branch1: 8x8 patches, d=8192, 576 key tokens) -> concat channels
  -> 3x3 conv + LeakyReLU(0.2).

Sharding: 8 cores = (video b in {0,1}) x (frame t in {0..3}). Each core
computes its full output frame [256,96,96]; K/V are recomputed per core
from its 4-frame video slice (no collectives). Host rotates frames so
xv[0] is the core's own frame; P columns and V tokens both use processed
order, so attention math is order-invariant.

All matmuls are bf16 with fp32 PSUM accumulation. x loads via SWDGE
cast-DMA (f32 DRAM -> bf16 SBUF); a bf16 copy of x is stashed to DRAM
during the K loop and re-read by the two V passes. Window gathers are a
handful of big multi-dim strided copies (Q's gather is folded into the
conv PSUM evacuation). V^T tiles are frame-aligned: br0 = 5 tiles/frame
(last 64 tokens short), br1 = 1 full tile/frame + one shared spill tile
(16 tokens/frame at partition offset f*32). Attention outputs stay in
SBUF (bf16) through the 3x3 conv.
"""

import sys

if "/opt/trn_rl_repo" not in sys.path:
    sys.path.insert(0, "/opt/trn_rl_repo")

import math
from contextlib import ExitStack

import numpy as np

import concourse.bass as bass
import concourse.tile as tile
from concourse import bacc, mybir
from concourse.masks import make_identity

F32 = mybir.dt.float32
F32R = mybir.dt.float32r
BF16 = mybir.dt.bfloat16

T = 4
C = 256
H = W = 96
PIX = H * W
NCORES = 8

PSZ = [4, 8]
OHB = [24, 12]                  # token grid side per branch
NTF = [576, 144]                # real tokens per frame
NTFP = [640, 144]               # P-column stride per frame
NKP = [2560, 640]               # key-token tiles * 128 per video
NQ = [576, 144]                 # query tokens (one frame)
NCH = [16, 64]                  # d-chunks (psz^2)
SC = [1.0 / math.sqrt(2048.0), 1.0 / math.sqrt(8192.0)]
NQB = [[(0, 128), (128, 128), (256, 128), (384, 128), (512, 64)],
       [(0, 128), (128, 16)]]

Exp = mybir.ActivationFunctionType.Exp
Identity = mybir.ActivationFunctionType.Identity


def build(nc):
    xv = nc.dram_tensor("xv", [T, C, PIX], F32R, kind="ExternalInput")
    wqt = nc.dram_tensor("wqt", [C, C], F32R, kind="ExternalInput")
    wkt = nc.dram_tensor("wkt", [C, C], F32R, kind="ExternalInput")
    wvt = nc.dram_tensor("wvt", [C, C], F32R, kind="ExternalInput")
    wot = nc.dram_tensor("wot", [9, C, C], F32R, kind="ExternalInput")
    bq = nc.dram_tensor("bq", [C], F32, kind="ExternalInput")
    bk = nc.dram_tensor("bk", [C], F32, kind="ExternalInput")
    bv = nc.dram_tensor("bv", [C], F32, kind="ExternalInput")
    bo = nc.dram_tensor("bo", [C], F32, kind="ExternalInput")
    out = nc.dram_tensor("out", [C, PIX], F32, kind="ExternalOutput")

    alt = [0]

    def evac_alt(dst, src, bias_ap=None):
        """PSUM -> SBUF evacuation, alternating scalar/vector engines."""
        alt[0] ^= 1
        if bias_ap is not None:
            if alt[0]:
                nc.scalar.activation(out=dst, in_=src, func=Identity,
                                     bias=bias_ap, scale=1.0)
            else:
                nc.vector.tensor_scalar_add(dst, src, bias_ap)
        else:
            if alt[0]:
                nc.scalar.copy(dst, src)
            else:
                nc.vector.tensor_copy(dst, src)

    galt = [0]

    def gather_alt(dst, src):
        galt[0] = (galt[0] + 1) % 3
        if galt[0] == 0:
            nc.vector.tensor_copy(dst, src)
        elif galt[0] == 1:
            nc.scalar.copy(dst, src)
        else:
            nc.gpsimd.tensor_copy(dst, src)

    with tile.TileContext(nc, pool_alloc_mode="queue") as tc, ExitStack() as top:
        persist = top.enter_context(tc.tile_pool(name="persist", bufs=1))
        dramp = top.enter_context(tc.tile_pool(name="dram", bufs=1,
                                               space="DRAM"))

        # bf16 weights: HWDGE f32 load + vector cast (keeps the SWDGE
        # queue free for the first x frame load)
        wq_sb, wk_sb, wv_sb = [None, None], [None, None], [None, None]
        with tc.tile_pool(name="wld", bufs=2) as p_wld:
            for name, dt_, lst in (("wk", wkt, wk_sb), ("wq", wqt, wq_sb),
                                   ("wv", wvt, wv_sb)):
                for cb in range(2):
                    f = p_wld.tile([128, C], F32R, name="wf", tag="wf")
                    nc.sync.dma_start(
                        out=f, in_=dt_.ap()[cb * 128:(cb + 1) * 128, :])
                    t = persist.tile([128, C], BF16, name=f"{name}{cb}",
                                     tag=f"{name}{cb}")
                    nc.vector.tensor_copy(t, f)
                    lst[cb] = t

        def bias_tile(name, dt_):
            t = persist.tile([128, 2], F32, tag=name)
            nc.sync.dma_start(
                out=t, in_=bass.AP(tensor=dt_.ap().tensor, offset=0,
                                   ap=[[1, 128], [128, 2]]))
            return t

        bq_sb = bias_tile("bq", bq)
        bk_sb = bias_tile("bk", bk)
        bv_sb = bias_tile("bv", bv)
        bo_sb = bias_tile("bo", bo)
        ident = persist.tile([128, 128], BF16, name="ident", tag="ident")
        make_identity(nc, ident)
        zrow = persist.tile([128, 98], BF16, name="zrow", tag="zrow")
        nc.vector.memset(zrow, 0.0)

        # bf16 stash of x in DRAM, written during the K loop
        xbf_d = [[dramp.tile([128, PIX], BF16, name=f"xd{j}{cb}",
                             tag=f"xd{j}{cb}") for cb in range(2)]
                 for j in range(T)]

        # Pool open order = reverse close order (LIFO):
        #   att (lives to end of D) < PT1 (to end of PV1) < PT0 (to end
        #   of PV0) < P (to end of transposes) < qw (to end of scores).
        # Tiles are created lazily at first use.
        esAtt = ExitStack()
        p_att = esAtt.enter_context(tc.tile_pool(name="att", bufs=1))
        esPT1 = ExitStack()
        p_PT1 = esPT1.enter_context(tc.tile_pool(name="PT1", bufs=1))
        esPT0 = ExitStack()
        p_PT0 = esPT0.enter_context(tc.tile_pool(name="PT0", bufs=1))

        # ---------------- phase A: per-frame QK conv + scores ------------
        esP = ExitStack()
        p_P = esP.enter_context(tc.tile_pool(name="P", bufs=1))
        p_t = [[p_P.tile([128, NKP[b]], BF16, name=f"p{b}_{i}",
                         tag=f"p{b}_{i}")
                for i in range(len(NQB[b]))] for b in range(2)]
        for b in range(2):
            for i in range(len(NQB[b])):
                nc.vector.memset(p_t[b][i][:, :], 0.0)

        esQW = ExitStack()
        p_qw = esQW.enter_context(tc.tile_pool(name="qw", bufs=1))
        qw = [p_qw.tile([128, NCH[b] * NTF[b]], BF16, name=f"qw{b}",
                        tag=f"qw{b}") for b in range(2)]
        p_run = esQW.enter_context(tc.tile_pool(name="run", bufs=1))
        run_mx = [[p_run.tile([128, 1], F32, name=f"mx{b}_{i}",
                              tag=f"mx{b}_{i}")
                   for i in range(len(NQB[b]))] for b in range(2)]
        run_ls = [[p_run.tile([128, 1], F32, name=f"ls{b}_{i}",
                              tag=f"ls{b}_{i}")
                   for i in range(len(NQB[b]))] for b in range(2)]
        p_stat = esQW.enter_context(tc.tile_pool(name="stat", bufs=4))

        ext = [[0 for _ in NQB[b]] for b in range(2)]   # rescale extent

        def conv_win(xcol, w_sb, b_sb, dst, ps_pool):
            """1x1 conv, PSUM evacuated straight into token-major layout
            dst[b][p, ci*ntf + tok] for both branches (+bias).
            xcol(cb, o) returns the x AP slice [128, 384] at pixel col o."""
            # branch0 (coutb 0): one psum region per token row (384 pix)
            d0 = dst[0].rearrange("p (wy wx oh ow) -> p wy wx oh ow",
                                  wy=4, wx=4, oh=24)
            for g in range(12):               # 2 token rows per psum
                ps = ps_pool.tile([128, 1024], F32, name="cps", tag="cps")
                for half in range(2):
                    oh = g * 2 + half
                    for cb in range(2):
                        nc.tensor.matmul(
                            ps[:, half * 512:half * 512 + 384],
                            w_sb[cb][:, 0:128],
                            xcol(cb, oh * 384),
                            start=(cb == 0), stop=(cb == 1))
                for half in range(2):
                    oh = g * 2 + half
                    src = ps[:, half * 512:half * 512 + 384].rearrange(
                        "p (wy ow wx) -> p wy wx ow", wy=4, ow=24)
                    evac_alt(d0[:, :, :, oh], src, b_sb[:, 0:1])
            # branch1 (coutb 1): half a token row (4 of 8 wy) per region
            d1 = dst[1].rearrange("p (wy wx oh ow) -> p wy wx oh ow",
                                  wy=8, wx=8, oh=12)
            for g in range(12):
                ps = ps_pool.tile([128, 1024], F32, name="cps", tag="cps")
                for half in range(2):
                    o = g * 768 + half * 384
                    for cb in range(2):
                        nc.tensor.matmul(
                            ps[:, half * 512:half * 512 + 384],
                            w_sb[cb][:, 128:256],
                            xcol(cb, o),
                            start=(cb == 0), stop=(cb == 1))
                for half in range(2):
                    oh, wyh = divmod(g * 2 + half, 2)
                    src = ps[:, half * 512:half * 512 + 384].rearrange(
                        "p (wy ow wx) -> p wy wx ow", wy=4, ow=12)
                    evac_alt(d1[:, wyh * 4:(wyh + 1) * 4, :, oh], src,
                             b_sb[:, 1:2])

        with tc.tile_pool(name="kx", bufs=1) as p_kx, \
             tc.tile_pool(name="kw", bufs=1) as p_kw, \
             tc.tile_pool(name="kps", bufs=2, space="PSUM") as p_kps, \
             tc.tile_pool(name="sps0", bufs=2, space="PSUM") as p_sps0, \
             tc.tile_pool(name="sps1", bufs=2, space="PSUM") as p_sps1:
            for j in range(T):
                xb = [[None, None], [None, None]]
                for hf in range(2):
                    for cb in range(2):
                        t = p_kx.tile([128, PIX // 2], BF16,
                                      name=f"kx{cb}{hf}", tag=f"kx{cb}{hf}")
                        nc.gpsimd.dma_start(
                            out=t,
                            in_=xv.ap()[j, cb * 128:(cb + 1) * 128,
                                        hf * 4608:(hf + 1) * 4608])
                        nc.sync.dma_start(
                            out=xbf_d[j][cb][:, hf * 4608:(hf + 1) * 4608],
                            in_=t)
                        xb[cb][hf] = t

                def xcol(cb, o, xb=xb):
                    hf, lo = divmod(o, 4608)
                    return xb[cb][hf][:, lo:lo + 384]

                kw = [p_kw.tile([128, NCH[b] * NTF[b]], BF16,
                                name=f"kw{b}", tag=f"kw{b}")
                      for b in range(2)]
                conv_win(xcol, wk_sb, bk_sb, kw, p_kps)
                if j == 0:
                    conv_win(xcol, wq_sb, bq_sb, qw, p_kps)

                # ---- scores for key frame j, both branches ----
                for b in range(2):
                    psz, ohb, ntf = PSZ[b], OHB[b], NTF[b]
                    nmk = 2 if b == 0 else 1
                    mkw = ntf // nmk              # 288 / 144
                    for nqi, (q0, nqsz) in enumerate(NQB[b]):
                        for mkh in range(nmk):
                            ps = (p_sps0 if b == 0 else p_sps1).tile(
                                [128, mkw], F32, name=f"sps{b}",
                                tag=f"sps{b}")
                            for ci in range(NCH[b]):
                                rhs = kw[b][:, ci * ntf + mkh * mkw:
                                            ci * ntf + (mkh + 1) * mkw]
                                lhsT = qw[b][:, ci * ntf + q0:
                                             ci * ntf + q0 + nqsz]
                                nc.tensor.matmul(
                                    ps[:nqsz], lhsT, rhs,
                                    start=(ci == 0),
                                    stop=(ci == NCH[b] - 1))
                            # online softmax over key blocks
                            o = j * NTFP[b] + mkh * mkw
                            pt = p_t[b][nqi]
                            mx, ls = run_mx[b][nqi], run_ls[b][nqi]
                            bm = p_stat.tile([128, 1], F32, name="bm",
                                             tag="bm")
                            nc.vector.reduce_max(out=bm[:nqsz],
                                                 in_=ps[:nqsz, :],
                                                 axis=mybir.AxisListType.X)
                            if j == 0 and mkh == 0:
                                nc.vector.tensor_copy(mx[:nqsz], bm[:nqsz])
                                nmx = p_stat.tile([128, 1], F32, name="nmx",
                                                  tag="nmx")
                                nc.vector.tensor_scalar_mul(
                                    nmx[:nqsz], mx[:nqsz], -SC[b])
                                nc.scalar.activation(
                                    out=pt[:nqsz, o:o + mkw],
                                    in_=ps[:nqsz, :], func=Exp,
                                    bias=nmx[:nqsz], scale=SC[b],
                                    accum_out=ls[:nqsz])
                            else:
                                nmax = p_stat.tile([128, 1], F32,
                                                   name="nmax", tag="nmax")
                                nc.vector.tensor_max(nmax[:nqsz], mx[:nqsz],
                                                     bm[:nqsz])
                                nmx = p_stat.tile([128, 1], F32, name="nmx",
                                                  tag="nmx")
                                nc.vector.tensor_scalar_mul(
                                    nmx[:nqsz], nmax[:nqsz], -SC[b])
                                delta = p_stat.tile([128, 1], F32,
                                                    name="delta",
                                                    tag="delta")
                                nc.scalar.activation(
                                    out=delta[:nqsz], in_=mx[:nqsz],
                                    func=Exp, bias=nmx[:nqsz], scale=SC[b])
                                e = ext[b][nqi]
                                nc.vector.tensor_scalar_mul(
                                    pt[:nqsz, 0:e], pt[:nqsz, 0:e],
                                    delta[:nqsz])
                                pl = p_stat.tile([128, 1], F32, name="pl",
                                                 tag="pl")
                                nc.scalar.activation(
                                    out=pt[:nqsz, o:o + mkw],
                                    in_=ps[:nqsz, :], func=Exp,
                                    bias=nmx[:nqsz], scale=SC[b],
                                    accum_out=pl[:nqsz])
                                nc.vector.scalar_tensor_tensor(
                                    out=ls[:nqsz], in0=ls[:nqsz],
                                    scalar=delta[:nqsz], in1=pl[:nqsz],
                                    op0=mybir.AluOpType.mult,
                                    op1=mybir.AluOpType.add)
                                nc.vector.tensor_copy(mx[:nqsz],
                                                      nmax[:nqsz])
                            ext[b][nqi] = max(ext[b][nqi], o + mkw)

        # final normalization of P
        for b in range(2):
            for nqi, (q0, nqsz) in enumerate(NQB[b]):
                rs = p_stat.tile([128, 1], F32, name="rs", tag="rs")
                nc.vector.reciprocal(rs[:nqsz], run_ls[b][nqi][:nqsz])
                nc.vector.tensor_scalar_mul(
                    p_t[b][nqi][:nqsz, :], p_t[b][nqi][:nqsz, :], rs[:nqsz])
        esQW.close()

        # ---------------- P^T transposes for both branches ----------------
        pt1_t = [p_PT1.tile([128, NQ[1]], BF16, name=f"pt1_{i}",
                            tag=f"pt1_{i}") for i in range(5)]
        nc.gpsimd.memset(pt1_t[4][:, :], 0.0)
        pt0_t = [p_PT0.tile([128, NQ[0]], BF16, name=f"pt0_{i}",
                            tag=f"pt0_{i}") for i in range(NKP[0] // 128)]

        with tc.tile_pool(name="ptps", bufs=4, space="PSUM") as p_ptps:
            for ti in range(NKP[0] // 128):
                for nqi, (q0, nqsz) in enumerate(NQB[0]):
                    tp = p_ptps.tile([128, 128], BF16, name="ptps",
                                     tag="ptps")
                    nc.tensor.transpose(
                        tp[:, :nqsz],
                        p_t[0][nqi][:nqsz, ti * 128:(ti + 1) * 128],
                        ident[:nqsz, :nqsz])
                    evac_alt(pt0_t[ti][:, q0:q0 + nqsz], tp[:, :nqsz])
            for f in range(T):
                for nqi, (q0, nqsz) in enumerate(NQB[1]):
                    tp = p_ptps.tile([128, 128], BF16, name="ptps",
                                     tag="ptps")
                    nc.tensor.transpose(
                        tp[:, :nqsz],
                        p_t[1][nqi][:nqsz, f * 144:f * 144 + 128],
                        ident[:nqsz, :nqsz])
                    evac_alt(pt1_t[f][:, q0:q0 + nqsz], tp[:, :nqsz])
                    tp2 = p_ptps.tile([128, 128], BF16, name="ptps2",
                                      tag="ptps2")
                    nc.tensor.transpose(
                        tp2[:16, :nqsz],
                        p_t[1][nqi][:nqsz, f * 144 + 128:(f + 1) * 144],
                        ident[:nqsz, :nqsz])
                    evac_alt(pt1_t[4][f * 32:f * 32 + 16, q0:q0 + nqsz],
                             tp2[:16, :nqsz])
        esP.close()

        # ---------------- att tiles (SBUF-resident, bf16) -----------------
        att_sb = {}

        def init_att(br):
            att = p_att.tile([128, 98 * 98], BF16, name=f"att{br}",
                             tag=f"att{br}")
            att_sb[br] = att
            attv = att.rearrange("p (h w) -> p h w", h=98)
            nc.gpsimd.tensor_copy(att[:, 0:98], zrow)
            nc.gpsimd.tensor_copy(att[:, 97 * 98:98 * 98], zrow)
            zcol = zrow[:, 0:96].rearrange("p (a c) -> p a c", a=96)
            nc.gpsimd.tensor_copy(attv[:, 1:97, 0:1], zcol)
            nc.gpsimd.tensor_copy(attv[:, 1:97, 97:98], zcol)
            return attv

        # ---------------- phase C: V build + PV, per branch ----------------
        for br in range(2):
            psz, ohb, ntf = PSZ[br], OHB[br], NTF[br]
            ntiles = NKP[br] // 128
            esV = ExitStack()
            p_V = esV.enter_context(tc.tile_pool(name=f"V{br}", bufs=1))
            v_t = [p_V.tile([128, NCH[br] * 128], BF16, name=f"v{br}_{i}",
                            tag=f"v{br}_{i}") for i in range(ntiles)]
            if br == 0:
                for f in range(T):
                    nc.gpsimd.memset(v_t[5 * f + 4][64:128, :], 0.0)
            else:
                nc.gpsimd.memset(v_t[4][:, :], 0.0)

            with tc.tile_pool(name=f"vx{br}", bufs=2) as p_vx, \
                 tc.tile_pool(name=f"xw{br}", bufs=1) as p_xw, \
                 tc.tile_pool(name=f"vps{br}", bufs=2,
                              space="PSUM") as p_vps:
                nquart = 4
                csz = PIX // nquart
                ohc = ohb // nquart
                for j in range(T):
                    xw = [p_xw.tile([128, NCH[br] * ntf], BF16,
                                    name=f"xw{cb}", tag=f"xw{cb}")
                          for cb in range(2)]
                    d5 = [xw[cb].rearrange(
                        "p (wy wx oh ow) -> p wy wx oh ow",
                        wy=psz, wx=psz, oh=ohb) for cb in range(2)]
                    for q in range(nquart):
                        for cb in range(2):
                            xt = p_vx.tile([128, csz], BF16,
                                           name=f"vx{cb}", tag=f"vx{cb}")
                            nc.sync.dma_start(
                                out=xt,
                                in_=xbf_d[j][cb][:, q * csz:(q + 1) * csz])
                            s5 = xt.rearrange(
                                "p (oh wy ow wx) -> p wy wx oh ow",
                                oh=ohc, wy=psz, ow=ohb)
                            for wy in range(psz):
                                gather_alt(
                                    d5[cb][:, wy, :,
                                           q * ohc:(q + 1) * ohc],
                                    s5[:, wy])
                    # V matmuls for frame j
                    if br == 0:
                        for sub in range(5):
                            m = 128 if sub < 4 else 64
                            t0 = sub * 128
                            ti = 5 * j + sub
                            ps = p_vps.tile([128, 2048], F32, name="vps",
                                            tag="vps")
                            for ci in range(16):
                                for cb in range(2):
                                    lhsT = xw[cb][:, ci * ntf + t0:
                                                  ci * ntf + t0 + m]
                                    nc.tensor.matmul(
                                        ps[:m, ci * 128:(ci + 1) * 128],
                                        lhsT,
                                        wv_sb[cb][:, 0:128],
                                        start=(cb == 0),
                                        stop=(cb == 1))
                            evac_alt(v_t[ti][:m, :], ps[:m, :])
                    else:
                        for cg in range(4):
                            ps = p_vps.tile([128, 2048], F32, name="vps",
                                            tag="vps")
                            for cl in range(16):
                                ci = cg * 16 + cl
                                for cb in range(2):
                                    lhsT = xw[cb][:, ci * ntf:
                                                  ci * ntf + 128]
                                    nc.tensor.matmul(
                                        ps[:, cl * 128:(cl + 1) * 128],
                                        lhsT,
                                        wv_sb[cb][:, 128:256],
                                        start=(cb == 0),
                                        stop=(cb == 1))
                            evac_alt(v_t[j][:, cg * 2048:(cg + 1) * 2048],
                                     ps)
                        off = j * 32
                        for cg in range(4):
                            ps = p_vps.tile([128, 2048], F32, name="vps",
                                            tag="vps")
                            for cl in range(16):
                                ci = cg * 16 + cl
                                for cb in range(2):
                                    lhsT = xw[cb][:, ci * ntf + 128:
                                                  ci * ntf + 144]
                                    nc.tensor.matmul(
                                        ps[off:off + 16,
                                           cl * 128:(cl + 1) * 128],
                                        lhsT,
                                        wv_sb[cb][:, 128:256],
                                        start=(cb == 0),
                                        stop=(cb == 1),
                                        tile_position=(0, off))
                            evac_alt(
                                v_t[4][off:off + 16,
                                       cg * 2048:(cg + 1) * 2048],
                                ps[off:off + 16, :])

            # --- PV: y^T accumulated over all key tiles; write into att ---
            attv = init_att(br)
            wvw = attv[:, 1:97, 1:97].rearrange(
                "p (oh hh) (ow ww) -> p oh hh ow ww", hh=psz, ww=psz)
            pt_t = pt0_t if br == 0 else pt1_t
            nqh_n = 2 if br == 0 else 1
            nqw = NQ[br] // nqh_n
            ohq = ohb // nqh_n
            with tc.tile_pool(name=f"pvps{br}", bufs=4,
                              space="PSUM") as p_pvps:
                for ci in range(NCH[br]):
                    wy, wx = divmod(ci, psz)
                    for nqh in range(nqh_n):
                        ps = p_pvps.tile([128, nqw], F32, name="pvps",
                                         tag="pvps")
                        for ti in range(ntiles):
                            nc.tensor.matmul(
                                ps, v_t[ti][:, ci * 128:(ci + 1) * 128],
                                pt_t[ti][:, nqh * nqw:(nqh + 1) * nqw],
                                start=(ti == 0), stop=(ti == ntiles - 1))
                        dst = wvw[:, nqh * ohq:(nqh + 1) * ohq, wy, :, wx]
                        src = ps.rearrange("p (a c) -> p a c", a=ohq)
                        evac_alt(dst, src, bv_sb[:, br:br + 1])
            esV.close()
            if br == 0:
                esPT0.close()
        esPT1.close()

        # ---------------- phase D: 3x3 conv + LeakyReLU ----------------
        with tc.tile_pool(name="wot", bufs=1) as p_wot, \
             tc.tile_pool(name="dout", bufs=3) as p_do, \
             tc.tile_pool(name="dps", bufs=2, space="PSUM") as p_dps:
            wot_sb = []
            for cb in range(2):
                t = p_wot.tile([128, 9, C], BF16, name=f"wot{cb}",
                               tag=f"wot{cb}")
                nc.gpsimd.dma_start(
                    out=t,
                    in_=wot.ap()[:, cb * 128:(cb + 1) * 128, :].rearrange(
                        "t i o -> i t o"))
                wot_sb.append(t)
            attv2 = [att_sb[cb].rearrange("p (h w) -> p h w", h=98)
                     for cb in range(2)]
            for coutb in range(2):
                for g in range(6):            # groups of 4 row-groups
                    ps = p_dps.tile([128, 2048], F32, name="dps",
                                    tag="dps")
                    for cb in range(2):
                        for tap in range(9):
                            dy, dx = divmod(tap, 3)
                            lhsT = wot_sb[cb][:, tap,
                                              coutb * 128:(coutb + 1) * 128]
                            for rg_ in range(4):
                                rg = g * 4 + rg_
                                rhs = attv2[cb][:, rg * 4 + dy:
                                                rg * 4 + dy + 4,
                                                dx:dx + 96]
                                nc.tensor.matmul(
                                    ps[:, rg_ * 512:rg_ * 512 + 384],
                                    lhsT, rhs,
                                    start=(cb == 0 and tap == 0),
                                    stop=(cb == 1 and tap == 8))
                    psv = ps.rearrange("p (a c) -> p a c", a=4)[:, :, 0:384]
                    t1 = p_do.tile([128, 1536], F32, name="t1", tag="t1")
                    t1v = t1.rearrange("p (a c) -> p a c", a=4)
                    nc.scalar.activation(out=t1v, in_=psv, func=Identity,
                                         bias=bo_sb[:, coutb:coutb + 1],
                                         scale=1.0)
                    t2 = p_do.tile([128, 1536], F32, name="t2", tag="t2")
                    nc.vector.scalar_tensor_tensor(
                        out=t2, in0=t1, scalar=0.2, in1=t1,
                        op0=mybir.AluOpType.mult,
                        op1=mybir.AluOpType.max)
                    nc.sync.dma_start(
                        out=out.ap()[coutb * 128:(coutb + 1) * 128,
                                     g * 1536:(g + 1) * 1536],
                        in_=t2)
        esAtt.close()
    return nc


_CACHED = {}


def _get_nc():
    if "nc" not in _CACHED:
        nc = bacc.Bacc("TRN2", debug=False, target_bir_lowering=False)
        build(nc)
        nc.compile()
        _CACHED["nc"] = nc
    return _CACHED["nc"]


def make_in_maps(x, wq, bq_, wk, bk_, wv, bv_, wo, bo_):
    shared = {
        "wqt": np.ascontiguousarray(wq.T.astype(np.float32)),
        "wkt": np.ascontiguousarray(wk.T.astype(np.float32)),
        "wvt": np.ascontiguousarray(wv.T.astype(np.float32)),
        "wot": np.ascontiguousarray(
            wo.transpose(2, 3, 1, 0).reshape(9, C, C).astype(np.float32)),
        "bq": np.ascontiguousarray(bq_.astype(np.float32)),
        "bk": np.ascontiguousarray(bk_.astype(np.float32)),
        "bv": np.ascontiguousarray(bv_.astype(np.float32)),
        "bo": np.ascontiguousarray(bo_.astype(np.float32)),
    }
    x3 = np.ascontiguousarray(x.reshape(2 * T, C, PIX).astype(np.float32))
    in_maps = []
    for core in range(NCORES):
        v, f = divmod(core, T)
        # rotate so the core's own frame is xv[0]; P and V both use
        # processed order, so attention math is order-invariant.
        order = [f] + [k for k in range(T) if k != f]
        m = dict(shared)
        m["xv"] = np.ascontiguousarray(x3[[v * T + k for k in order]])
        in_maps.append(m)
    return in_maps


def kernel(**inputs):
    from concourse.bass_utils import run_bass_kernel_spmd

    x = np.asarray(inputs["x"], dtype=np.float32)
    in_maps = make_in_maps(
        x, np.asarray(inputs["wq"]), np.asarray(inputs["bq"]),
        np.asarray(inputs["wk"]), np.asarray(inputs["bk"]),
        np.asarray(inputs["wv"]), np.asarray(inputs["bv"]),
        np.asarray(inputs["wo"]), np.asarray(inputs["bo"]))
    nc = _get_nc()
    res = run_bass_kernel_spmd(nc, in_maps, core_ids=list(range(NCORES)))
    outs = [res.results[c]["out"].reshape(C, H, W) for c in range(NCORES)]
    return np.stack(outs).astype(np.float32)
